# revision 51
# baseline (speedup 1.0000x reference)
"""Multi-head attention (B=2, S=2048, D=1024, H=16) on 8 Trainium2 cores.

Sharding: core c -> (batch b = c//4, head-group g = c%4, 4 heads each).
Tensor-parallel over heads within a batch; the output projection is done
per head-group against the matching Wo column slice and the partial
[S, D] results are summed on the host (plus the folded biases bo + Wo@bv).

v3: PV matmuls run P-stationary (exp-scores bf16) with V bf16 as the
65-wide moving operand (64 dims + a ones column accumulating the softmax
denominator). Projections run fully in bf16 (halves the startup DMA);
scores and the output projection stay f32r<->f32r. Softmax normalization
is a batched per-partition reciprocal + broadcast multiply on DVE. The
normalized context is transposed back with PE identity-matmuls. Each
pair's last PV flushes + normalize are carried into the next pair's
first iterations so the Activation engine never drains at boundaries.
All DMAs issue from the SP / Pool queues.
"""

from contextlib import ExitStack

import numpy as np

import concourse.bacc as bacc
import concourse.tile as tile
from concourse import mybir

D_MODEL = 1024
NUM_HEADS = 16
D_K = 64
B = 2
S_FULL = 2048
N_CORES = 8
GH = 4              # heads per core
GJ = GH * D_K       # 256 columns per head-group

F32 = mybir.dt.float32
F32R = mybir.dt.float32r
BF16 = mybir.dt.bfloat16
I16 = mybir.dt.int16
AF = mybir.ActivationFunctionType
ALU = mybir.AluOpType

# Schraudolph-style exp on DVE: rint(ps*SCHR_A + SCHR_B) as int16, bitcast
# bf16 gives ~2^(ps/8*log2e) with a systematic multiplicative bias; the Act
# halves carry exp(ps/8 + BETA) so both halves share the bias, which then
# cancels in the softmax normalization.
SCHR_A = float(0.125 * 128 * np.log2(np.e))
SCHR_B = 16256.0 - 60.0
BETA = -0.28520


WARM_A = 9
WARM_B = 0
VF_LATE = False
EXP_P = 8
EXP_P_HF0 = 8
QJT_ON_ACT = True
EXP_MERGE = False
PT_BUFS = 9
TAKE3 = True
TRANSP_DMA = True
T_IN_CARRY = True
KQ_ON_ACT = False
OALLT_ON_ACT = False


def build_nc(S=S_FULL, SB=512):
    """Build + compile the per-core program (identical on all 8 cores)."""
    NCH = S // SB     # chunks (and hf blocks)
    ST = S // 128     # sk tiles
    DT = D_MODEL // 128
    JT = GJ // 128    # 2 j-tiles (2 heads each)
    STB = ST // NCH   # sk-tiles per chunk
    SQT = SB // 128   # sq 128-tiles per hf block

    nc = bacc.Bacc("TRN2", target_bir_lowering=False, debug=False)

    xqT = nc.dram_tensor("xqT", [NCH, 128, DT, SB], BF16, kind="ExternalInput").ap()
    xkT = nc.dram_tensor("xkT", [NCH, 128, DT, SB], BF16, kind="ExternalInput").ap()
    xvT = nc.dram_tensor("xvT", [NCH, 128, DT, SB], BF16, kind="ExternalInput").ap()
    wqT = nc.dram_tensor("wqT", [128, DT, GJ], BF16, kind="ExternalInput").ap()
    wkT = nc.dram_tensor("wkT", [128, DT, GJ], BF16, kind="ExternalInput").ap()
    wvT = nc.dram_tensor("wvT", [128, DT, GJ], BF16, kind="ExternalInput").ap()
    woT = nc.dram_tensor("woT", [128, GJ // 128, D_MODEL], BF16, kind="ExternalInput").ap()
    bq = nc.dram_tensor("bq", [128, GJ // 128], F32, kind="ExternalInput").ap()
    bk = nc.dram_tensor("bk", [128, GJ // 128], F32, kind="ExternalInput").ap()
    ident = nc.dram_tensor("ident", [128, 128], BF16, kind="ExternalInput").ap()
    yT = nc.dram_tensor("yT", [D_MODEL, S], BF16, kind="ExternalOutput").ap()

    with tile.TileContext(nc) as tc:
        with ExitStack() as ctx:
            cpool = ctx.enter_context(tc.tile_pool(name="const", bufs=1))
            xk_pool = ctx.enter_context(tc.tile_pool(name="xk", bufs=3))
            xq_pool = ctx.enter_context(tc.tile_pool(name="xq", bufs=2))
            xv_pool = ctx.enter_context(tc.tile_pool(name="xv", bufs=2))
            p_pool = ctx.enter_context(tc.tile_pool(name="pt",
                                                    bufs=PT_BUFS))
            y_pool = ctx.enter_context(tc.tile_pool(name="ys", bufs=4))
            s_pool = ctx.enter_context(tc.tile_pool(name="sm", bufs=4))
            ps_s = ctx.enter_context(tc.tile_pool(
                name="ps2", bufs=2 if EXP_MERGE else 4, space="PSUM"))

            def ps_tile(name):
                # burst tiles share the score-psum ring: with EXP_MERGE the
                # ring is 2 x [128, 2, SB] and a burst borrows a full tile
                if EXP_MERGE:
                    t = ps_s.tile([128, 2, SB], F32, tag="psm", name=name)
                    return t[:, 0, :]
                return ps_s.tile([128, SB], F32, tag="ps", name=name)[:]
            ps_b = ctx.enter_context(tc.tile_pool(name="pb2", bufs=1, space="PSUM"))

            # Four manually-scheduled 1-bank PSUM slots. The live pair's PV
            # accumulators hold one pair of slots ((A,B) or (C,D),
            # alternating per pair); transient tiles (transpose, out-proj,
            # q-proj) cycle on the opposite pair of slots.
            _pb_state = {"t": ("pbC", "pbD"), "i": 0}

            def pb_tile(name, tag):
                return ps_b.tile([128, 512], F32, tag=tag, name=name, bufs=1)

            def pb_next(name):
                tags = _pb_state["t"]
                tag = tags[_pb_state["i"] % len(tags)]
                _pb_state["i"] += 1
                return pb_tile(name, tag)

            # ---- persistent SBUF ----
            wq_sb = cpool.tile([128, DT, GJ], BF16, tag="wq")
            wk_sb = cpool.tile([128, DT, GJ], BF16, tag="wk")
            wv_sb = cpool.tile([128, DT, GJ], BF16, tag="wv")
            wo_sb = cpool.tile([128, JT, D_MODEL], BF16, tag="wo")
            bq_sb = cpool.tile([128, JT], F32, tag="bq")
            bk_sb = cpool.tile([128, JT], F32, tag="bk")
            id_sb = cpool.tile([128, 128], BF16, tag="ident")
            warm_sb = cpool.tile([128, 512], F32R, tag="warm")

            qhT_sb = cpool.tile([128, JT, S], F32R, tag="qhT")
            khT_sb = cpool.tile([128, JT, S], F32R, tag="khT")
            vh_sb = cpool.tile([128, ST, GH, 65], BF16, tag="vh")
            oall_sb = cpool.tile([128, ST, GJ], BF16, tag="oall")
            # layout [jp, hf, c, jt, q]: the XBAR transpose-DMA emits
            # 128-row blocks in source free-dim order (c major, jt minor),
            # which lands contiguously in this layout
            oallT_sb = cpool.tile([128, NCH, SQT, JT, 128], BF16,
                                  tag="oallT")

            ones_sb = cpool.tile([128, 1], F32, tag="ones")
            beta_sb = cpool.tile([128, 1], F32, tag="beta")

            # ---- initial DMAs ----
            # The DMA transfer resource and the HWDGE generator are both
            # serial, so the critical prefix goes on ONE hwdge queue (SP) in
            # strict priority order; tiny consts go via SWDGE (parallel
            # generator lane) so they interleave without stealing slots.
            xs_k0 = xk_pool.tile([128, DT, SB], BF16, tag="xk", name="xs_k0")
            xs_q0 = xq_pool.tile([128, DT, SB], BF16, tag="xq", name="xs_q0")
            xs_v0 = xv_pool.tile([128, DT, SB], BF16, tag="xv", name="xs_v0")
            xs_k1 = xk_pool.tile([128, DT, SB], BF16, tag="xk", name="xs_k1")
            xs_v1 = xv_pool.tile([128, DT, SB], BF16, tag="xv", name="xs_v1")
            nc.sync.dma_start(xs_k0[:, 0:4, :], xkT[0][:, 0:4, :])
            nc.sync.dma_start(wk_sb[:], wkT)
            nc.sync.dma_start(xs_k0[:, 4:8, :], xkT[0][:, 4:8, :])
            nc.sync.dma_start(xs_q0[:, 0:4, :], xqT[0][:, 0:4, :])
            nc.sync.dma_start(wq_sb[:], wqT)
            nc.sync.dma_start(xs_q0[:, 4:8, :], xqT[0][:, 4:8, :])
            nc.sync.dma_start(wv_sb[:], wvT)
            nc.sync.dma_start(xs_v0[:], xvT[0])
            nc.sync.dma_start(xs_k1[:], xkT[1])
            nc.sync.dma_start(xs_v1[:], xvT[1])
            nc.gpsimd.dma_start(bk_sb[:], bk)
            nc.gpsimd.dma_start(bq_sb[:], bq)
            nc.gpsimd.dma_start(id_sb[:], ident)

            # warmup: keep the PE busy (and p-state ramped) during the
            # initial DMA window; results are never read. Reads warm_sb
            # UNINITIALIZED on purpose (no dependency on the memset below) so
            # the PE starts immediately; the memset only has to land before
            # the first acc-reset matmul in stage B.
            def warm_mms(tag, n):
                for i in range(n):
                    wps = pb_tile(f"warm{tag}{i}",
                                  ("pbA", "pbB", "pbC", "pbD")[i % 4])
                    nc.tensor.matmul(
                        wps[:], warm_sb[:, 0:128], warm_sb[:],
                        start=True, stop=True,
                    )

            warm_mms("a", WARM_A)
            nc.vector.memset(warm_sb[:].bitcast(F32), 0.0)
            nc.vector.memset(ones_sb[:], 1.0)
            nc.vector.memset(beta_sb[:], BETA)
            nc.vector.tensor_copy(
                vh_sb[:, :, :, 64:65],
                ones_sb[:, None, :].broadcast_to([128, ST, GH, 1]),
            )

            # ---- projections ----
            def kq_proj(which, sb, xs, jts=(0, 1)):
                """Transposed projection chunk -> qhT/khT[:, jt, sb*SB:...]
                through a [128, 1024] ps_s scratch tile (pre-B / window
                boundaries only)."""
                w_sb, b_sb, outT = {
                    "k": (wk_sb, bk_sb, khT_sb),
                    "q": (wq_sb, bq_sb, qhT_sb),
                }[which]
                ss = slice(sb * SB, (sb + 1) * SB)
                for jt in jts:
                    ps = ps_tile(f"ps_{which}{sb}_{jt}")
                    for d in range(DT):
                        nc.tensor.matmul(
                            ps,
                            w_sb[:, d, jt * 128:(jt + 1) * 128],
                            xs[:, d, :],
                            start=(d == 0),
                            stop=(d == DT - 1),
                        )
                    if KQ_ON_ACT:
                        nc.scalar.activation(outT[:, jt, ss], ps, AF.Identity,
                                             bias=b_sb[:, jt:jt + 1])
                    else:
                        nc.vector.tensor_scalar_add(
                            outT[:, jt, ss], ps, b_sb[:, jt:jt + 1]
                        )

            def v_proj(sb, xs, stls=(0, 1, 2, 3)):
                """Normal-layout projection chunk -> vh (bf16, ones kept)."""
                ps = ps_tile(f"ps_v{sb}_{stls[0]}")
                for i, stl in enumerate(stls):
                    st = sb * (SB // 128) + stl
                    sl = slice(i * GJ, (i + 1) * GJ)
                    for d in range(DT):
                        nc.tensor.matmul(
                            ps[:, sl],
                            xs[:, d, stl * 128:(stl + 1) * 128],
                            wv_sb[:, d, :],
                            start=(d == 0),
                            stop=(d == DT - 1),
                        )
                    nc.scalar.activation(
                        vh_sb[:, st, :, 0:64],
                        ps[:, sl].rearrange("p (h e) -> p h e", h=GH),
                        AF.Copy,
                    )

            def q_jt_units(sb, xs, jt):
                """q-projection of one j-tile as 4 filler units (2 matmuls
                each) accumulating into a transient pb slot."""
                cell = {}
                units = []
                for g in range(4):
                    def u(g=g):
                        if g == 0:
                            cell["t"] = pb_next(f"qp_{sb}_{jt}")
                        t = cell["t"]
                        for d in (2 * g, 2 * g + 1):
                            nc.tensor.matmul(
                                t[:],
                                wq_sb[:, d, jt * 128:(jt + 1) * 128],
                                xs[:, d, :],
                                start=(d == 0),
                                stop=(d == DT - 1),
                            )
                        if g == 3:
                            if QJT_ON_ACT:
                                nc.scalar.activation(
                                    qhT_sb[:, jt, sb * SB:(sb + 1) * SB],
                                    t[:], AF.Identity,
                                    bias=bq_sb[:, jt:jt + 1])
                            else:
                                nc.vector.tensor_scalar_add(
                                    qhT_sb[:, jt, sb * SB:(sb + 1) * SB],
                                    t[:], bq_sb[:, jt:jt + 1]
                                )
                    units.append(u)
                return units

            # ---- stage B: scores -> exp -> PV (P-stationary) ----
            acc_live = {}
            pend_live = {}

            def pv_mms(hf, hp, st, pt):
                # st==0/c==0 carries start=True: it marks the whole PSUM bank
                # pending-zero, so no separate acc-reset matmul is needed
                accs = acc_live[(hf, hp)]
                for hl in range(2):
                    acc = accs[hl]
                    for c in range(SQT):
                        nc.tensor.matmul(
                            acc[:, c, 0:65],
                            pt[:, hl * SB + c * 128:hl * SB + (c + 1) * 128],
                            vh_sb[:, st, 2 * hp + hl, :],
                            start=(st == 0 and c == 0),
                            stop=(st == ST - 1),
                            skip_group_check=True,
                        )

            def finish_units(hf, hp):
                """Last PV flushes + softmax normalize of a finished pair,
                as schedulable units (run inside the NEXT pair's loop)."""
                units = []
                for st, pt in pend_live.pop((hf, hp)):
                    units.append(
                        lambda st=st, pt=pt: pv_mms(hf, hp, st, pt))

                def norm():
                    accs = acc_live.pop((hf, hp))
                    for hl in range(2):
                        rcp = s_pool.tile([128, SQT], F32, tag="rcp",
                                          name=f"rcp_{hf}_{hp}_{hl}")
                        nc.vector.reciprocal(
                            rcp[:],
                            accs[hl][:, :, 64:65].rearrange("p c e -> p (c e)"),
                        )
                        nc.vector.tensor_mul(
                            oall_sb[:, hf * SQT:(hf + 1) * SQT,
                                    (2 * hp + hl) * 64:(2 * hp + hl + 1) * 64],
                            accs[hl][:, :, 0:64],
                            rcp[:, :, None].broadcast_to([128, SQT, 64]),
                        )
                units.append(norm)
                return units

            def b_pair(hf, hp, st_lo, st_hi, fillers=None, acc_tags=None,
                       carry=None, lag=2):
                hs = slice(hf * SB, (hf + 1) * SB)
                jt = hp
                if carry:
                    # first own-PV must follow the carried normalize of the
                    # pair whose accumulator slots we reclaim
                    lag = max(lag, len(carry))

                def ensure_accs():
                    if (hf, hp) in acc_live:
                        return
                    accs = []
                    for hl in range(2):
                        t = pb_tile(f"acc_{hf}_{hp}_{hl}", acc_tags[hl])
                        accs.append(t.rearrange("p (c e) -> p c e", c=SQT))
                    acc_live[(hf, hp)] = accs
                # PV runs two iterations behind scores so its exp dependency
                # is always satisfied by the time the PE reaches it
                pend = pend_live.pop((hf, hp), [])
                started = (hf, hp) in acc_live
                nst = st_hi - st_lo
                for st in range(st_lo, st_hi):
                    pss = []
                    if EXP_MERGE:
                        psm = ps_s.tile([128, 2, SB], F32, tag="psm",
                                        name=f"psb_{hf}_{hp}_{st}")
                    for hl in range(2):
                        base = 64 * hl
                        if EXP_MERGE:
                            psh = psm[:, hl, :]
                        else:
                            psh = ps_tile(f"psb_{hf}_{hp}_{st}_{hl}")
                        nc.tensor.matmul(
                            psh,
                            khT_sb[base:base + 64, jt, st * 128:(st + 1) * 128],
                            qhT_sb[base:base + 64, jt, hs],
                            start=True, stop=True,
                        )
                        pss.append(psh)
                    # spread remaining fillers over the iterations left (late
                    # iterations are exp-supply-gated, so PE filler work there
                    # hides the wait)
                    if carry:
                        carry.pop(0)()
                    elif fillers and len(fillers) >= st_hi - st:
                        fillers.pop(0)()
                    elif fillers and (st - st_lo) % 2 == 1:
                        fillers.pop(0)()
                    if started or len(pend) >= lag:
                        started = True
                        take = 2 if len(pend) > 2 else (
                            1 if len(pend) >= 2 else 0)
                        if TAKE3 and len(pend) > 4:
                            take = 3
                        for _ in range(take):
                            ensure_accs()
                            pv_mms(hf, hp, *pend.pop(0))
                    pt = p_pool.tile([128, JT * SB], BF16, tag="pt",
                                     name=f"pt_{hf}_{hp}_{st}")
                    expp = EXP_P if hf > 0 else EXP_P_HF0
                    if EXP_MERGE:
                        dve = st % 2 == 1 and (expp == 0 or st % expp != 1)
                        if not dve:
                            nc.scalar.activation(
                                pt[:].rearrange("p (h s) -> p h s", h=2),
                                psm[:], AF.Exp, scale=0.125,
                                bias=beta_sb[:, 0:1])
                        else:
                            nc.vector.tensor_scalar(
                                pt[:].bitcast(I16).rearrange(
                                    "p (h s) -> p h s", h=2),
                                psm[:], SCHR_A, SCHR_B, ALU.mult, ALU.add)
                    else:
                        for hl in range(2):
                            sl = slice(hl * SB, (hl + 1) * SB)
                            dve = (st + hl) % 2 == 1 and (expp == 0
                                                          or st % expp != 1)
                            if not dve:
                                nc.scalar.activation(pt[:, sl], pss[hl],
                                                     AF.Exp, scale=0.125,
                                                     bias=beta_sb[:, 0:1])
                            else:
                                nc.vector.tensor_scalar(pt[:, sl].bitcast(I16),
                                                        pss[hl],
                                                        SCHR_A, SCHR_B,
                                                        ALU.mult, ALU.add)
                    pend.append((st, pt))
                pend_live[(hf, hp)] = pend

            # ---- stage C: transpose + output projection ----
            # y DMAs batch 4 mt-tiles per transfer (one issue each) via the
            # [128, 4, SB] staging tiles; DRAM side is viewed partition-major
            # to match the SBUF AP iteration order.
            yg = yT.rearrange("(g t p) s -> g p t s", t=4, p=128)

            def t_units(hf, use_dma=False):
                """Transpose oall[sq, gj] -> oallT[gj, sq] for one hf block.
                use_dma: one XBAR transpose-DMA for the whole block (no PE
                or DVE time, but ~2.3us latency - keep off the tail)."""
                if use_dma:
                    def u():
                        nc.sync.dma_start_transpose(
                            oallT_sb[:, hf].rearrange("p c j q -> p (c j) q"),
                            oall_sb[:, hf * SQT:(hf + 1) * SQT, :]
                            .rearrange("p c g -> p (c g)"),
                        )
                    return [u]
                units = []
                for jt2 in range(JT):
                    def u(jt2=jt2):
                        tp = pb_next(f"tp_{hf}_{jt2}")
                        tpb = tp[:, 0:256].bitcast(BF16)
                        for c in range(SQT):
                            nc.tensor.transpose(
                                tpb[:, c * 128:(c + 1) * 128],
                                oall_sb[:, hf * SQT + c,
                                        jt2 * 128:(jt2 + 1) * 128],
                                id_sb[:],
                            )
                        nc.vector.tensor_copy(
                            oallT_sb[:, hf, :, jt2, :],
                            tpb[:].rearrange("p (c q) -> p c q", c=SQT))
                    units.append(u)
                return units

            def c_units(hf, tail=False):
                hs = slice(hf * SB, (hf + 1) * SB)
                units = []
                yts = {}
                for mt in range(DT):
                    def u(mt=mt):
                        pc = pb_next(f"pc_{hf}_{mt}")
                        for kt in range(JT):
                            nc.tensor.matmul(
                                pc[:],
                                wo_sb[:, kt, mt * 128:(mt + 1) * 128],
                                oallT_sb[:, hf, :, kt, :],
                                start=(kt == 0),
                                stop=(kt == JT - 1),
                            )
                        # tail groups are 2 mt-tiles so the final DMA chain
                        # starts as early as possible; mid-stream groups are 4
                        gsz = 2 if tail else 4
                        g = mt // gsz
                        if mt % gsz == 0:
                            yts[g] = y_pool.tile([128, gsz, SB], BF16,
                                                 tag="yt",
                                                 name=f"yt_{hf}_{g}")
                        yt = yts[g]
                        if mt % 2:
                            nc.vector.tensor_copy(yt[:, mt % gsz, :], pc[:])
                        else:
                            nc.scalar.activation(yt[:, mt % gsz, :], pc[:],
                                                 AF.Copy)
                        if mt % gsz == gsz - 1:
                            eng = nc.sync if (tail or g == 0) else nc.gpsimd
                            ygv = yg if not tail else yT.rearrange(
                                "(g t p) s -> g p t s", t=2, p=128)
                            eng.dma_start(ygv[g][:, :, hs], yt[:])
                    units.append(u)
                return units

            def q_load(sb, eng):
                xs = xq_pool.tile([128, DT, SB], BF16, tag="xq",
                                  name=f"xs_qf{sb}")
                eng.dma_start(xs[:], xqT[sb])
                return xs

            # ---- fused schedule ----
            # Pre-B order matches the serial DMA arrival order (k chunk+wk
            # first, then q chunk+wq); v_proj(0) runs late in sb0 when xv0
            # has landed.
            kq_proj("k", 0, xs_k0, jts=(0,))
            kq_proj("k", 0, xs_k0, jts=(1,))
            warm_mms("b", WARM_B)
            kq_proj("q", 0, xs_q0, jts=(0,))
            kq_proj("q", 0, xs_q0, jts=(1,))

            # hf0: both pairs interleaved chunk-wise (PV accumulators occupy
            # all four pb slots); chunk sb+1's k/v projections issue in small
            # bursts at window boundaries so the Act backlog covers them.
            xs_k = {0: xs_k0, 1: xs_k1}
            xs_v = {0: xs_v0, 1: xs_v1}
            q_next = None
            for sb in range(NCH):
                if sb + 2 < NCH:
                    xs_k[sb + 2] = xk_pool.tile([128, DT, SB], BF16, tag="xk",
                                                name=f"xs_k{sb + 2}")
                    nc.sync.dma_start(xs_k[sb + 2][:], xkT[sb + 2])
                    xs_v[sb + 2] = xv_pool.tile([128, DT, SB], BF16, tag="xv",
                                                name=f"xs_v{sb + 2}")
                    nc.gpsimd.dma_start(xs_v[sb + 2][:], xvT[sb + 2])
                if sb + 1 < NCH:
                    bursts = [
                        lambda: kq_proj("k", sb + 1, xs_k[sb + 1], jts=(0,)),
                        lambda: kq_proj("k", sb + 1, xs_k[sb + 1], jts=(1,)),
                        lambda: v_proj(sb + 1, xs_v[sb + 1], stls=(0, 1)),
                        lambda: v_proj(sb + 1, xs_v[sb + 1], stls=(2, 3)),
                    ]
                else:
                    bursts = []
                if sb == 1:
                    nc.gpsimd.dma_start(wo_sb[:], woT)
                if sb == 2:
                    q_next = q_load(1, nc.sync)
                if sb == 3:
                    bursts = [
                        lambda: kq_proj("q", 1, q_next, jts=(0,)),
                        lambda: kq_proj("q", 1, q_next, jts=(1,)),
                    ]
                if sb == 0:
                    # no PVs in pair0's first windows (v not projected yet);
                    # the PE units here are ordered to match the serial DMA
                    # arrivals: q-jt1 after xq0b, v_proj(0) after xv0, the
                    # chunk-1 bursts after xk1/xv1
                    vf = [lambda: v_proj(0, xs_v0, stls=(0, 1)),
                          lambda: v_proj(0, xs_v0, stls=(2, 3))]
                    b_pair(0, 0, 0, 2, acc_tags=("pbA", "pbB"), lag=99)
                    b_pair(0, 1, 0, 2, acc_tags=("pbC", "pbD"), lag=99)
                    b_pair(0, 0, 2, 4, acc_tags=("pbA", "pbB"), lag=99)
                    if VF_LATE:
                        b_pair(0, 1, 2, 4, acc_tags=("pbC", "pbD"))
                        vf[0]()
                        vf[1]()
                    else:
                        vf[0]()
                        vf[1]()
                        b_pair(0, 1, 2, 4, acc_tags=("pbC", "pbD"))
                    bursts[0]()
                    bursts[1]()
                    bursts[2]()
                    bursts[3]()
                    continue
                # half-windows: one ~1.7us projection burst per boundary so
                # the Act backlog covers each
                H = STB // 2
                for half in range(2):
                    lo = sb * STB + half * H
                    b_pair(0, 0, lo, lo + H, acc_tags=("pbA", "pbB"))
                    if bursts:
                        bursts.pop(0)()
                    b_pair(0, 1, lo, lo + H, acc_tags=("pbC", "pbD"))
                    if bursts:
                        bursts.pop(0)()

            # hf1..3: the live pair's accumulators alternate (A,B)/(C,D);
            # transients cycle on the opposite slots. Each pair's tail (PV
            # flush + normalize) is carried into the next pair's loop.
            carry = finish_units(0, 0) + finish_units(0, 1)
            tag_flip = {("pbA", "pbB"): ("pbC", "pbD"),
                        ("pbC", "pbD"): ("pbA", "pbB")}
            cur_tags = ("pbC", "pbD")
            for hf in range(1, NCH):
                fillers = []
                if hf + 1 < NCH:
                    q_next = q_load(hf + 1, nc.gpsimd if hf % 2 else nc.sync)
                    fillers += q_jt_units(hf + 1, q_next, 0)
                    fillers += q_jt_units(hf + 1, q_next, 1)
                tus = t_units(hf - 1, use_dma=TRANSP_DMA)
                if TRANSP_DMA and T_IN_CARRY:
                    # after the carried normalize (which writes oall[hf-1]),
                    # but before the regular fillers
                    carry = carry + tus
                    fillers += c_units(hf - 1)
                else:
                    fillers += tus + c_units(hf - 1)
                if hf == NCH - 1:
                    tp_last = t_units(hf)
                for hp in range(2):
                    _pb_state["t"] = cur_tags  # transients on the free pair
                    use = tag_flip[cur_tags]
                    if hf == NCH - 1 and hp == 1:
                        fillers.append(tp_last[0])
                    b_pair(hf, hp, 0, ST, fillers, acc_tags=use, carry=carry)
                    carry = finish_units(hf, hp)
                    cur_tags = use
                for u in fillers:
                    u()
            for u in carry:
                u()
            _pb_state["t"] = ("pbA", "pbB", "pbC", "pbD")
            _pb_state["i"] = 0
            tp_last[1]()
            for u in c_units(NCH - 1, tail=True):
                u()

    nc.compile()
    return nc


_NC_CACHE = {}


def _get_nc(S=S_FULL):
    if S not in _NC_CACHE:
        _NC_CACHE[S] = build_nc(S)
    return _NC_CACHE[S]


def make_in_maps(q, k, v, Wq, bq, Wk, bk, Wv, bv, Wo, bo, S=S_FULL):
    import ml_dtypes

    q = np.asarray(q, np.float32)
    k = np.asarray(k, np.float32)
    v = np.asarray(v, np.float32)
    Wq = np.asarray(Wq, np.float32)
    Wk = np.asarray(Wk, np.float32)
    Wv = np.asarray(Wv, np.float32)
    Wo = np.asarray(Wo, np.float32)
    bq = np.asarray(bq, np.float32)
    bk = np.asarray(bk, np.float32)

    SB = 512
    NCH = S // SB
    DT = D_MODEL // 128
    bf16 = ml_dtypes.bfloat16
    eye = np.ascontiguousarray(np.eye(128, dtype=np.float32).astype(bf16))

    def xtile(x):
        # [S, D] -> xT [D, S] -> [NCH, 128, DT, SB]: t[sb, p, d, s] = x[sb*SB+s, d*128+p]
        xT = x.T  # [D, S]
        return np.ascontiguousarray(
            xT.reshape(DT, 128, NCH, SB).transpose(2, 1, 0, 3).astype(bf16)
        )

    def wtile(wT):
        # [D, GJ] -> [128, DT, GJ]
        return np.ascontiguousarray(
            wT.reshape(DT, 128, GJ).transpose(1, 0, 2).astype(bf16))

    in_maps = []
    for c in range(N_CORES):
        b, g = divmod(c, GH)
        sl = slice(g * GJ, (g + 1) * GJ)
        woT = Wo[:, sl].T  # [GJ, D]
        in_maps.append({
            "xqT": xtile(q[b, :S]),
            "xkT": xtile(k[b, :S]),
            "xvT": xtile(v[b, :S]),
            "wqT": wtile(Wq[sl].T),
            "wkT": wtile(Wk[sl].T),
            "wvT": wtile(Wv[sl].T),
            "woT": np.ascontiguousarray(
                woT.reshape(2, 128, D_MODEL).transpose(1, 0, 2).astype(bf16)
            ),
            "bq": np.ascontiguousarray(bq[sl].reshape(2, 128).T),
            "bk": np.ascontiguousarray(bk[sl].reshape(2, 128).T),
            "ident": eye,
        })
    return in_maps


def gather_out(results, Wo, bv, bo, S=S_FULL):
    Wo = np.asarray(Wo, np.float32)
    bv = np.asarray(bv, np.float32)
    bo = np.asarray(bo, np.float32)
    out = np.zeros((B, S, D_MODEL), np.float32)
    for c in range(N_CORES):
        out[c // GH] += results[c]["yT"].astype(np.float32).T
    out += bo + Wo @ bv
    return out


def kernel(q, k, v, Wq, bq, Wk, bk, Wv, bv, Wo, bo):
    from concourse.bass_utils import run_bass_kernel_spmd

    nc = _get_nc(S_FULL)
    in_maps = make_in_maps(q, k, v, Wq, bq, Wk, bk, Wv, bv, Wo, bo)
    res = run_bass_kernel_spmd(nc, in_maps, core_ids=list(range(N_CORES)))
    return gather_out(res.results, Wo, bv, bo)



# revision 52
# speedup vs baseline: 1.0006x; 1.0006x over previous
"""Multi-head attention (B=2, S=2048, D=1024, H=16) on 8 Trainium2 cores.

Sharding: core c -> (batch b = c//4, head-group g = c%4, 4 heads each).
Tensor-parallel over heads within a batch; the output projection is done
per head-group against the matching Wo column slice and the partial
[S, D] results are summed on the host (plus the folded biases bo + Wo@bv).

v4 (from the v3 baseline): the softmax exp is split per head-half
across TWO engines - Act computes exact exp(ps/8 + BETA) halves, DVE
computes Schraudolph bit-trick halves (rint(ps*A + B) as int16, bitcast
bf16); both halves carry the same multiplicative bias, which cancels in
the softmax normalization (rel err ~1.24e-2). Score PSUM tiles are one
bank per head-half (4-deep ring) so each exp releases its bank ~0.6us
after the score matmul - the PE never stalls on PSUM reuse. PV
accumulators skip the explicit reset matmul (st0/c0 carries start=True,
which marks the whole bank pending-zero). Context transposes for hf0-2
are single XBAR transpose-DMAs into a block-contiguous oallT layout;
only the last block stays on the PE+DVE path to keep the tail short.
Wo / the context are bf16, y output DMAs are batched 4 (mid) / 2 (tail)
tiles per transfer, and the initial DMA queue order matches the serial
DMA resource so the projection head streams without stalls.
"""

from contextlib import ExitStack

import numpy as np

import concourse.bacc as bacc
import concourse.tile as tile
from concourse import mybir

D_MODEL = 1024
NUM_HEADS = 16
D_K = 64
B = 2
S_FULL = 2048
N_CORES = 8
GH = 4              # heads per core
GJ = GH * D_K       # 256 columns per head-group

F32 = mybir.dt.float32
F32R = mybir.dt.float32r
BF16 = mybir.dt.bfloat16
I16 = mybir.dt.int16
AF = mybir.ActivationFunctionType
ALU = mybir.AluOpType

# Schraudolph-style exp on DVE: rint(ps*SCHR_A + SCHR_B) as int16, bitcast
# bf16 gives ~2^(ps/8*log2e) with a systematic multiplicative bias; the Act
# halves carry exp(ps/8 + BETA) so both halves share the bias, which then
# cancels in the softmax normalization.
SCHR_A = float(0.125 * 128 * np.log2(np.e))
SCHR_B = 16256.0 - 60.0
BETA = -0.28520


WARM_A = 9
WARM_B = 0
VF_LATE = False
EXP_P = 8
EXP_P_HF0 = 8
QJT_ON_ACT = True
EXP_MERGE = False
PT_BUFS = 9
TAKE3 = True
TRANSP_DMA = True
T_IN_CARRY = False
KQ_ON_ACT = False
OALLT_ON_ACT = False


def build_nc(S=S_FULL, SB=512):
    """Build + compile the per-core program (identical on all 8 cores)."""
    NCH = S // SB     # chunks (and hf blocks)
    ST = S // 128     # sk tiles
    DT = D_MODEL // 128
    JT = GJ // 128    # 2 j-tiles (2 heads each)
    STB = ST // NCH   # sk-tiles per chunk
    SQT = SB // 128   # sq 128-tiles per hf block

    nc = bacc.Bacc("TRN2", target_bir_lowering=False, debug=False)

    xqT = nc.dram_tensor("xqT", [NCH, 128, DT, SB], BF16, kind="ExternalInput").ap()
    xkT = nc.dram_tensor("xkT", [NCH, 128, DT, SB], BF16, kind="ExternalInput").ap()
    xvT = nc.dram_tensor("xvT", [NCH, 128, DT, SB], BF16, kind="ExternalInput").ap()
    wqT = nc.dram_tensor("wqT", [128, DT, GJ], BF16, kind="ExternalInput").ap()
    wkT = nc.dram_tensor("wkT", [128, DT, GJ], BF16, kind="ExternalInput").ap()
    wvT = nc.dram_tensor("wvT", [128, DT, GJ], BF16, kind="ExternalInput").ap()
    woT = nc.dram_tensor("woT", [128, GJ // 128, D_MODEL], BF16, kind="ExternalInput").ap()
    bq = nc.dram_tensor("bq", [128, GJ // 128], F32, kind="ExternalInput").ap()
    bk = nc.dram_tensor("bk", [128, GJ // 128], F32, kind="ExternalInput").ap()
    ident = nc.dram_tensor("ident", [128, 128], BF16, kind="ExternalInput").ap()
    yT = nc.dram_tensor("yT", [D_MODEL, S], BF16, kind="ExternalOutput").ap()

    with tile.TileContext(nc) as tc:
        with ExitStack() as ctx:
            cpool = ctx.enter_context(tc.tile_pool(name="const", bufs=1))
            xk_pool = ctx.enter_context(tc.tile_pool(name="xk", bufs=3))
            xq_pool = ctx.enter_context(tc.tile_pool(name="xq", bufs=2))
            xv_pool = ctx.enter_context(tc.tile_pool(name="xv", bufs=2))
            p_pool = ctx.enter_context(tc.tile_pool(name="pt",
                                                    bufs=PT_BUFS))
            y_pool = ctx.enter_context(tc.tile_pool(name="ys", bufs=4))
            s_pool = ctx.enter_context(tc.tile_pool(name="sm", bufs=4))
            ps_s = ctx.enter_context(tc.tile_pool(
                name="ps2", bufs=2 if EXP_MERGE else 4, space="PSUM"))

            def ps_tile(name):
                # burst tiles share the score-psum ring: with EXP_MERGE the
                # ring is 2 x [128, 2, SB] and a burst borrows a full tile
                if EXP_MERGE:
                    t = ps_s.tile([128, 2, SB], F32, tag="psm", name=name)
                    return t[:, 0, :]
                return ps_s.tile([128, SB], F32, tag="ps", name=name)[:]
            ps_b = ctx.enter_context(tc.tile_pool(name="pb2", bufs=1, space="PSUM"))

            # Four manually-scheduled 1-bank PSUM slots. The live pair's PV
            # accumulators hold one pair of slots ((A,B) or (C,D),
            # alternating per pair); transient tiles (transpose, out-proj,
            # q-proj) cycle on the opposite pair of slots.
            _pb_state = {"t": ("pbC", "pbD"), "i": 0}

            def pb_tile(name, tag):
                return ps_b.tile([128, 512], F32, tag=tag, name=name, bufs=1)

            def pb_next(name):
                tags = _pb_state["t"]
                tag = tags[_pb_state["i"] % len(tags)]
                _pb_state["i"] += 1
                return pb_tile(name, tag)

            # ---- persistent SBUF ----
            wq_sb = cpool.tile([128, DT, GJ], BF16, tag="wq")
            wk_sb = cpool.tile([128, DT, GJ], BF16, tag="wk")
            wv_sb = cpool.tile([128, DT, GJ], BF16, tag="wv")
            wo_sb = cpool.tile([128, JT, D_MODEL], BF16, tag="wo")
            bq_sb = cpool.tile([128, JT], F32, tag="bq")
            bk_sb = cpool.tile([128, JT], F32, tag="bk")
            id_sb = cpool.tile([128, 128], BF16, tag="ident")
            warm_sb = cpool.tile([128, 512], F32R, tag="warm")

            qhT_sb = cpool.tile([128, JT, S], F32R, tag="qhT")
            khT_sb = cpool.tile([128, JT, S], F32R, tag="khT")
            vh_sb = cpool.tile([128, ST, GH, 65], BF16, tag="vh")
            oall_sb = cpool.tile([128, ST, GJ], BF16, tag="oall")
            # layout [jp, hf, c, jt, q]: the XBAR transpose-DMA emits
            # 128-row blocks in source free-dim order (c major, jt minor),
            # which lands contiguously in this layout
            oallT_sb = cpool.tile([128, NCH, SQT, JT, 128], BF16,
                                  tag="oallT")

            ones_sb = cpool.tile([128, 1], F32, tag="ones")
            beta_sb = cpool.tile([128, 1], F32, tag="beta")

            # ---- initial DMAs ----
            # The DMA transfer resource and the HWDGE generator are both
            # serial, so the critical prefix goes on ONE hwdge queue (SP) in
            # strict priority order; tiny consts go via SWDGE (parallel
            # generator lane) so they interleave without stealing slots.
            xs_k0 = xk_pool.tile([128, DT, SB], BF16, tag="xk", name="xs_k0")
            xs_q0 = xq_pool.tile([128, DT, SB], BF16, tag="xq", name="xs_q0")
            xs_v0 = xv_pool.tile([128, DT, SB], BF16, tag="xv", name="xs_v0")
            xs_k1 = xk_pool.tile([128, DT, SB], BF16, tag="xk", name="xs_k1")
            xs_v1 = xv_pool.tile([128, DT, SB], BF16, tag="xv", name="xs_v1")
            nc.sync.dma_start(xs_k0[:, 0:4, :], xkT[0][:, 0:4, :])
            nc.sync.dma_start(wk_sb[:], wkT)
            nc.sync.dma_start(xs_k0[:, 4:8, :], xkT[0][:, 4:8, :])
            nc.sync.dma_start(xs_q0[:, 0:4, :], xqT[0][:, 0:4, :])
            nc.sync.dma_start(wq_sb[:], wqT)
            nc.sync.dma_start(xs_q0[:, 4:8, :], xqT[0][:, 4:8, :])
            nc.sync.dma_start(wv_sb[:], wvT)
            nc.sync.dma_start(xs_v0[:], xvT[0])
            nc.sync.dma_start(xs_k1[:], xkT[1])
            nc.sync.dma_start(xs_v1[:], xvT[1])
            nc.gpsimd.dma_start(bk_sb[:], bk)
            nc.gpsimd.dma_start(bq_sb[:], bq)
            nc.gpsimd.dma_start(id_sb[:], ident)

            # warmup: keep the PE busy (and p-state ramped) during the
            # initial DMA window; results are never read. Reads warm_sb
            # UNINITIALIZED on purpose (no dependency on the memset below) so
            # the PE starts immediately; the memset only has to land before
            # the first acc-reset matmul in stage B.
            def warm_mms(tag, n):
                for i in range(n):
                    wps = pb_tile(f"warm{tag}{i}",
                                  ("pbA", "pbB", "pbC", "pbD")[i % 4])
                    nc.tensor.matmul(
                        wps[:], warm_sb[:, 0:128], warm_sb[:],
                        start=True, stop=True,
                    )

            warm_mms("a", WARM_A)
            nc.vector.memset(warm_sb[:].bitcast(F32), 0.0)
            nc.vector.memset(ones_sb[:], 1.0)
            nc.vector.memset(beta_sb[:], BETA)
            nc.vector.tensor_copy(
                vh_sb[:, :, :, 64:65],
                ones_sb[:, None, :].broadcast_to([128, ST, GH, 1]),
            )

            # ---- projections ----
            def kq_proj(which, sb, xs, jts=(0, 1)):
                """Transposed projection chunk -> qhT/khT[:, jt, sb*SB:...]
                through a [128, 1024] ps_s scratch tile (pre-B / window
                boundaries only)."""
                w_sb, b_sb, outT = {
                    "k": (wk_sb, bk_sb, khT_sb),
                    "q": (wq_sb, bq_sb, qhT_sb),
                }[which]
                ss = slice(sb * SB, (sb + 1) * SB)
                for jt in jts:
                    ps = ps_tile(f"ps_{which}{sb}_{jt}")
                    for d in range(DT):
                        nc.tensor.matmul(
                            ps,
                            w_sb[:, d, jt * 128:(jt + 1) * 128],
                            xs[:, d, :],
                            start=(d == 0),
                            stop=(d == DT - 1),
                        )
                    if KQ_ON_ACT:
                        nc.scalar.activation(outT[:, jt, ss], ps, AF.Identity,
                                             bias=b_sb[:, jt:jt + 1])
                    else:
                        nc.vector.tensor_scalar_add(
                            outT[:, jt, ss], ps, b_sb[:, jt:jt + 1]
                        )

            def v_proj(sb, xs, stls=(0, 1, 2, 3)):
                """Normal-layout projection chunk -> vh (bf16, ones kept)."""
                ps = ps_tile(f"ps_v{sb}_{stls[0]}")
                for i, stl in enumerate(stls):
                    st = sb * (SB // 128) + stl
                    sl = slice(i * GJ, (i + 1) * GJ)
                    for d in range(DT):
                        nc.tensor.matmul(
                            ps[:, sl],
                            xs[:, d, stl * 128:(stl + 1) * 128],
                            wv_sb[:, d, :],
                            start=(d == 0),
                            stop=(d == DT - 1),
                        )
                    nc.scalar.activation(
                        vh_sb[:, st, :, 0:64],
                        ps[:, sl].rearrange("p (h e) -> p h e", h=GH),
                        AF.Copy,
                    )

            def q_jt_units(sb, xs, jt):
                """q-projection of one j-tile as 4 filler units (2 matmuls
                each) accumulating into a transient pb slot."""
                cell = {}
                units = []
                for g in range(4):
                    def u(g=g):
                        if g == 0:
                            cell["t"] = pb_next(f"qp_{sb}_{jt}")
                        t = cell["t"]
                        for d in (2 * g, 2 * g + 1):
                            nc.tensor.matmul(
                                t[:],
                                wq_sb[:, d, jt * 128:(jt + 1) * 128],
                                xs[:, d, :],
                                start=(d == 0),
                                stop=(d == DT - 1),
                            )
                        if g == 3:
                            if QJT_ON_ACT:
                                nc.scalar.activation(
                                    qhT_sb[:, jt, sb * SB:(sb + 1) * SB],
                                    t[:], AF.Identity,
                                    bias=bq_sb[:, jt:jt + 1])
                            else:
                                nc.vector.tensor_scalar_add(
                                    qhT_sb[:, jt, sb * SB:(sb + 1) * SB],
                                    t[:], bq_sb[:, jt:jt + 1]
                                )
                    units.append(u)
                return units

            # ---- stage B: scores -> exp -> PV (P-stationary) ----
            acc_live = {}
            pend_live = {}

            def pv_mms(hf, hp, st, pt):
                # st==0/c==0 carries start=True: it marks the whole PSUM bank
                # pending-zero, so no separate acc-reset matmul is needed
                accs = acc_live[(hf, hp)]
                for hl in range(2):
                    acc = accs[hl]
                    for c in range(SQT):
                        nc.tensor.matmul(
                            acc[:, c, 0:65],
                            pt[:, hl * SB + c * 128:hl * SB + (c + 1) * 128],
                            vh_sb[:, st, 2 * hp + hl, :],
                            start=(st == 0 and c == 0),
                            stop=(st == ST - 1),
                            skip_group_check=True,
                        )

            def finish_units(hf, hp):
                """Last PV flushes + softmax normalize of a finished pair,
                as schedulable units (run inside the NEXT pair's loop)."""
                units = []
                for st, pt in pend_live.pop((hf, hp)):
                    units.append(
                        lambda st=st, pt=pt: pv_mms(hf, hp, st, pt))

                def norm():
                    accs = acc_live.pop((hf, hp))
                    for hl in range(2):
                        rcp = s_pool.tile([128, SQT], F32, tag="rcp",
                                          name=f"rcp_{hf}_{hp}_{hl}")
                        nc.vector.reciprocal(
                            rcp[:],
                            accs[hl][:, :, 64:65].rearrange("p c e -> p (c e)"),
                        )
                        nc.vector.tensor_mul(
                            oall_sb[:, hf * SQT:(hf + 1) * SQT,
                                    (2 * hp + hl) * 64:(2 * hp + hl + 1) * 64],
                            accs[hl][:, :, 0:64],
                            rcp[:, :, None].broadcast_to([128, SQT, 64]),
                        )
                units.append(norm)
                return units

            def b_pair(hf, hp, st_lo, st_hi, fillers=None, acc_tags=None,
                       carry=None, lag=2):
                hs = slice(hf * SB, (hf + 1) * SB)
                jt = hp
                if carry:
                    # first own-PV must follow the carried normalize of the
                    # pair whose accumulator slots we reclaim
                    lag = max(lag, len(carry))

                def ensure_accs():
                    if (hf, hp) in acc_live:
                        return
                    accs = []
                    for hl in range(2):
                        t = pb_tile(f"acc_{hf}_{hp}_{hl}", acc_tags[hl])
                        accs.append(t.rearrange("p (c e) -> p c e", c=SQT))
                    acc_live[(hf, hp)] = accs
                # PV runs two iterations behind scores so its exp dependency
                # is always satisfied by the time the PE reaches it
                pend = pend_live.pop((hf, hp), [])
                started = (hf, hp) in acc_live
                nst = st_hi - st_lo
                for st in range(st_lo, st_hi):
                    pss = []
                    if EXP_MERGE:
                        psm = ps_s.tile([128, 2, SB], F32, tag="psm",
                                        name=f"psb_{hf}_{hp}_{st}")
                    for hl in range(2):
                        base = 64 * hl
                        if EXP_MERGE:
                            psh = psm[:, hl, :]
                        else:
                            psh = ps_tile(f"psb_{hf}_{hp}_{st}_{hl}")
                        nc.tensor.matmul(
                            psh,
                            khT_sb[base:base + 64, jt, st * 128:(st + 1) * 128],
                            qhT_sb[base:base + 64, jt, hs],
                            start=True, stop=True,
                        )
                        pss.append(psh)
                    # spread remaining fillers over the iterations left (late
                    # iterations are exp-supply-gated, so PE filler work there
                    # hides the wait)
                    if carry:
                        carry.pop(0)()
                    elif fillers and len(fillers) >= st_hi - st:
                        fillers.pop(0)()
                    elif fillers and (st - st_lo) % 2 == 1:
                        fillers.pop(0)()
                    if started or len(pend) >= lag:
                        started = True
                        take = 2 if len(pend) > 2 else (
                            1 if len(pend) >= 2 else 0)
                        if TAKE3 and len(pend) > 4:
                            take = 3
                        for _ in range(take):
                            ensure_accs()
                            pv_mms(hf, hp, *pend.pop(0))
                    pt = p_pool.tile([128, JT * SB], BF16, tag="pt",
                                     name=f"pt_{hf}_{hp}_{st}")
                    expp = EXP_P if hf > 0 else EXP_P_HF0
                    if EXP_MERGE:
                        dve = st % 2 == 1 and (expp == 0 or st % expp != 1)
                        if not dve:
                            nc.scalar.activation(
                                pt[:].rearrange("p (h s) -> p h s", h=2),
                                psm[:], AF.Exp, scale=0.125,
                                bias=beta_sb[:, 0:1])
                        else:
                            nc.vector.tensor_scalar(
                                pt[:].bitcast(I16).rearrange(
                                    "p (h s) -> p h s", h=2),
                                psm[:], SCHR_A, SCHR_B, ALU.mult, ALU.add)
                    else:
                        for hl in range(2):
                            sl = slice(hl * SB, (hl + 1) * SB)
                            dve = (st + hl) % 2 == 1 and (expp == 0
                                                          or st % expp != 1)
                            if not dve:
                                nc.scalar.activation(pt[:, sl], pss[hl],
                                                     AF.Exp, scale=0.125,
                                                     bias=beta_sb[:, 0:1])
                            else:
                                nc.vector.tensor_scalar(pt[:, sl].bitcast(I16),
                                                        pss[hl],
                                                        SCHR_A, SCHR_B,
                                                        ALU.mult, ALU.add)
                    pend.append((st, pt))
                pend_live[(hf, hp)] = pend

            # ---- stage C: transpose + output projection ----
            # y DMAs batch 4 mt-tiles per transfer (one issue each) via the
            # [128, 4, SB] staging tiles; DRAM side is viewed partition-major
            # to match the SBUF AP iteration order.
            yg = yT.rearrange("(g t p) s -> g p t s", t=4, p=128)

            def t_units(hf, use_dma=False):
                """Transpose oall[sq, gj] -> oallT[gj, sq] for one hf block.
                use_dma: one XBAR transpose-DMA for the whole block (no PE
                or DVE time, but ~2.3us latency - keep off the tail)."""
                if use_dma:
                    def u():
                        nc.sync.dma_start_transpose(
                            oallT_sb[:, hf].rearrange("p c j q -> p (c j) q"),
                            oall_sb[:, hf * SQT:(hf + 1) * SQT, :]
                            .rearrange("p c g -> p (c g)"),
                        )
                    return [u]
                units = []
                for jt2 in range(JT):
                    def u(jt2=jt2):
                        tp = pb_next(f"tp_{hf}_{jt2}")
                        tpb = tp[:, 0:256].bitcast(BF16)
                        for c in range(SQT):
                            nc.tensor.transpose(
                                tpb[:, c * 128:(c + 1) * 128],
                                oall_sb[:, hf * SQT + c,
                                        jt2 * 128:(jt2 + 1) * 128],
                                id_sb[:],
                            )
                        nc.vector.tensor_copy(
                            oallT_sb[:, hf, :, jt2, :],
                            tpb[:].rearrange("p (c q) -> p c q", c=SQT))
                    units.append(u)
                return units

            def c_units(hf, tail=False):
                hs = slice(hf * SB, (hf + 1) * SB)
                units = []
                yts = {}
                for mt in range(DT):
                    def u(mt=mt):
                        pc = pb_next(f"pc_{hf}_{mt}")
                        for kt in range(JT):
                            nc.tensor.matmul(
                                pc[:],
                                wo_sb[:, kt, mt * 128:(mt + 1) * 128],
                                oallT_sb[:, hf, :, kt, :],
                                start=(kt == 0),
                                stop=(kt == JT - 1),
                            )
                        # tail groups are 2 mt-tiles so the final DMA chain
                        # starts as early as possible; mid-stream groups are 4
                        gsz = 2 if tail else 4
                        g = mt // gsz
                        if mt % gsz == 0:
                            yts[g] = y_pool.tile([128, gsz, SB], BF16,
                                                 tag="yt",
                                                 name=f"yt_{hf}_{g}")
                        yt = yts[g]
                        if mt % 2:
                            nc.vector.tensor_copy(yt[:, mt % gsz, :], pc[:])
                        else:
                            nc.scalar.activation(yt[:, mt % gsz, :], pc[:],
                                                 AF.Copy)
                        if mt % gsz == gsz - 1:
                            eng = nc.sync if (tail or g == 0) else nc.gpsimd
                            ygv = yg if not tail else yT.rearrange(
                                "(g t p) s -> g p t s", t=2, p=128)
                            eng.dma_start(ygv[g][:, :, hs], yt[:])
                    units.append(u)
                return units

            def q_load(sb, eng):
                xs = xq_pool.tile([128, DT, SB], BF16, tag="xq",
                                  name=f"xs_qf{sb}")
                eng.dma_start(xs[:], xqT[sb])
                return xs

            # ---- fused schedule ----
            # Pre-B order matches the serial DMA arrival order (k chunk+wk
            # first, then q chunk+wq); v_proj(0) runs late in sb0 when xv0
            # has landed.
            kq_proj("k", 0, xs_k0, jts=(0,))
            kq_proj("k", 0, xs_k0, jts=(1,))
            warm_mms("b", WARM_B)
            kq_proj("q", 0, xs_q0, jts=(0,))
            kq_proj("q", 0, xs_q0, jts=(1,))

            # hf0: both pairs interleaved chunk-wise (PV accumulators occupy
            # all four pb slots); chunk sb+1's k/v projections issue in small
            # bursts at window boundaries so the Act backlog covers them.
            xs_k = {0: xs_k0, 1: xs_k1}
            xs_v = {0: xs_v0, 1: xs_v1}
            q_next = None
            for sb in range(NCH):
                if sb + 2 < NCH:
                    xs_k[sb + 2] = xk_pool.tile([128, DT, SB], BF16, tag="xk",
                                                name=f"xs_k{sb + 2}")
                    nc.sync.dma_start(xs_k[sb + 2][:], xkT[sb + 2])
                    xs_v[sb + 2] = xv_pool.tile([128, DT, SB], BF16, tag="xv",
                                                name=f"xs_v{sb + 2}")
                    nc.gpsimd.dma_start(xs_v[sb + 2][:], xvT[sb + 2])
                if sb + 1 < NCH:
                    bursts = [
                        lambda: kq_proj("k", sb + 1, xs_k[sb + 1], jts=(0,)),
                        lambda: kq_proj("k", sb + 1, xs_k[sb + 1], jts=(1,)),
                        lambda: v_proj(sb + 1, xs_v[sb + 1], stls=(0, 1)),
                        lambda: v_proj(sb + 1, xs_v[sb + 1], stls=(2, 3)),
                    ]
                else:
                    bursts = []
                if sb == 1:
                    nc.gpsimd.dma_start(wo_sb[:], woT)
                if sb == 2:
                    q_next = q_load(1, nc.sync)
                if sb == 3:
                    bursts = [
                        lambda: kq_proj("q", 1, q_next, jts=(0,)),
                        lambda: kq_proj("q", 1, q_next, jts=(1,)),
                    ]
                if sb == 0:
                    # no PVs in pair0's first windows (v not projected yet);
                    # the PE units here are ordered to match the serial DMA
                    # arrivals: q-jt1 after xq0b, v_proj(0) after xv0, the
                    # chunk-1 bursts after xk1/xv1
                    vf = [lambda: v_proj(0, xs_v0, stls=(0, 1)),
                          lambda: v_proj(0, xs_v0, stls=(2, 3))]
                    b_pair(0, 0, 0, 2, acc_tags=("pbA", "pbB"), lag=99)
                    b_pair(0, 1, 0, 2, acc_tags=("pbC", "pbD"), lag=99)
                    b_pair(0, 0, 2, 4, acc_tags=("pbA", "pbB"), lag=99)
                    if VF_LATE:
                        b_pair(0, 1, 2, 4, acc_tags=("pbC", "pbD"))
                        vf[0]()
                        vf[1]()
                    else:
                        vf[0]()
                        vf[1]()
                        b_pair(0, 1, 2, 4, acc_tags=("pbC", "pbD"))
                    bursts[0]()
                    bursts[1]()
                    bursts[2]()
                    bursts[3]()
                    continue
                # half-windows: one ~1.7us projection burst per boundary so
                # the Act backlog covers each
                H = STB // 2
                for half in range(2):
                    lo = sb * STB + half * H
                    b_pair(0, 0, lo, lo + H, acc_tags=("pbA", "pbB"))
                    if bursts:
                        bursts.pop(0)()
                    b_pair(0, 1, lo, lo + H, acc_tags=("pbC", "pbD"))
                    if bursts:
                        bursts.pop(0)()

            # hf1..3: the live pair's accumulators alternate (A,B)/(C,D);
            # transients cycle on the opposite slots. Each pair's tail (PV
            # flush + normalize) is carried into the next pair's loop.
            carry = finish_units(0, 0) + finish_units(0, 1)
            tag_flip = {("pbA", "pbB"): ("pbC", "pbD"),
                        ("pbC", "pbD"): ("pbA", "pbB")}
            cur_tags = ("pbC", "pbD")
            for hf in range(1, NCH):
                fillers = []
                if hf + 1 < NCH:
                    q_next = q_load(hf + 1, nc.gpsimd if hf % 2 else nc.sync)
                    fillers += q_jt_units(hf + 1, q_next, 0)
                    fillers += q_jt_units(hf + 1, q_next, 1)
                tus = t_units(hf - 1, use_dma=TRANSP_DMA)
                if TRANSP_DMA and T_IN_CARRY:
                    # after the carried normalize (which writes oall[hf-1]),
                    # but before the regular fillers
                    carry = carry + tus
                    fillers += c_units(hf - 1)
                else:
                    fillers += tus + c_units(hf - 1)
                if hf == NCH - 1:
                    tp_last = t_units(hf)
                for hp in range(2):
                    _pb_state["t"] = cur_tags  # transients on the free pair
                    use = tag_flip[cur_tags]
                    if hf == NCH - 1 and hp == 1:
                        fillers.append(tp_last[0])
                    b_pair(hf, hp, 0, ST, fillers, acc_tags=use, carry=carry)
                    carry = finish_units(hf, hp)
                    cur_tags = use
                for u in fillers:
                    u()
            for u in carry:
                u()
            _pb_state["t"] = ("pbA", "pbB", "pbC", "pbD")
            _pb_state["i"] = 0
            tp_last[1]()
            for u in c_units(NCH - 1, tail=True):
                u()

    nc.compile()
    return nc


_NC_CACHE = {}


def _get_nc(S=S_FULL):
    if S not in _NC_CACHE:
        _NC_CACHE[S] = build_nc(S)
    return _NC_CACHE[S]


def make_in_maps(q, k, v, Wq, bq, Wk, bk, Wv, bv, Wo, bo, S=S_FULL):
    import ml_dtypes

    q = np.asarray(q, np.float32)
    k = np.asarray(k, np.float32)
    v = np.asarray(v, np.float32)
    Wq = np.asarray(Wq, np.float32)
    Wk = np.asarray(Wk, np.float32)
    Wv = np.asarray(Wv, np.float32)
    Wo = np.asarray(Wo, np.float32)
    bq = np.asarray(bq, np.float32)
    bk = np.asarray(bk, np.float32)

    SB = 512
    NCH = S // SB
    DT = D_MODEL // 128
    bf16 = ml_dtypes.bfloat16
    eye = np.ascontiguousarray(np.eye(128, dtype=np.float32).astype(bf16))

    def xtile(x):
        # [S, D] -> xT [D, S] -> [NCH, 128, DT, SB]: t[sb, p, d, s] = x[sb*SB+s, d*128+p]
        xT = x.T  # [D, S]
        return np.ascontiguousarray(
            xT.reshape(DT, 128, NCH, SB).transpose(2, 1, 0, 3).astype(bf16)
        )

    def wtile(wT):
        # [D, GJ] -> [128, DT, GJ]
        return np.ascontiguousarray(
            wT.reshape(DT, 128, GJ).transpose(1, 0, 2).astype(bf16))

    in_maps = []
    for c in range(N_CORES):
        b, g = divmod(c, GH)
        sl = slice(g * GJ, (g + 1) * GJ)
        woT = Wo[:, sl].T  # [GJ, D]
        in_maps.append({
            "xqT": xtile(q[b, :S]),
            "xkT": xtile(k[b, :S]),
            "xvT": xtile(v[b, :S]),
            "wqT": wtile(Wq[sl].T),
            "wkT": wtile(Wk[sl].T),
            "wvT": wtile(Wv[sl].T),
            "woT": np.ascontiguousarray(
                woT.reshape(2, 128, D_MODEL).transpose(1, 0, 2).astype(bf16)
            ),
            "bq": np.ascontiguousarray(bq[sl].reshape(2, 128).T),
            "bk": np.ascontiguousarray(bk[sl].reshape(2, 128).T),
            "ident": eye,
        })
    return in_maps


def gather_out(results, Wo, bv, bo, S=S_FULL):
    Wo = np.asarray(Wo, np.float32)
    bv = np.asarray(bv, np.float32)
    bo = np.asarray(bo, np.float32)
    out = np.zeros((B, S, D_MODEL), np.float32)
    for c in range(N_CORES):
        out[c // GH] += results[c]["yT"].astype(np.float32).T
    out += bo + Wo @ bv
    return out


def kernel(q, k, v, Wq, bq, Wk, bk, Wv, bv, Wo, bo):
    from concourse.bass_utils import run_bass_kernel_spmd

    nc = _get_nc(S_FULL)
    in_maps = make_in_maps(q, k, v, Wq, bq, Wk, bk, Wv, bv, Wo, bo)
    res = run_bass_kernel_spmd(nc, in_maps, core_ids=list(range(N_CORES)))
    return gather_out(res.results, Wo, bv, bo)

